# revision 10
# baseline (speedup 1.0000x reference)
"""Causal multi-head attention (B=4, S=2048, D=1024, H=16) on 8 trn2 cores.

Sharding: tensor-parallel over heads — 2 heads per core. Every core reads the
full x (pre-transposed on host) and its own column-slices of Wq/Wk/Wv plus the
matching row-slice of Wo, computes a partial output [rows_valid, D], and the
host sums the 8 partials (+bo) and scatters into the padded [B, S, D] output.

Per-core kernel (all matmuls fp32r — full PE rate at N>=256, ~1e-4 rel err):
  proj:  qT/kT/vT [128, L] = Wt.T @ xT   (weights stationary, x moving)
  vaug:  PE-transpose vT -> v natural [k, 64] per head, plus a ones column
         (softmax denominator folds into the AV matmul for free)
  attn:  per q-tile (512) x key-chunk (128), both heads emitted adjacent
         (K=64 scores matmuls land in distinct PE row-groups -> concurrent):
         scoresT[k,q] -> exp (ACT, no max subtraction; scores are O(1))
         -> mask the single diagonal 128x128 block -> AV accumulate
         ctxT[65, q] in PSUM, row 64 = softmax denominator.
  norm:  recip(denominator) -> gpsimd partition_broadcast -> multiply
  outp:  out[rows, D] partial = ctxT.T @ Wo_slice -> DMA out

To keep the PE HAM clock-gate at K=8/8 (2.4 GHz), emission interleaves dense
PE work into the attention phase: out-projection row-tiles are emitted right
after the q-tile that produces their ctxT columns, and batch b+1's projection
/ v-transpose units are round-robined between attention key-chunks.
"""

import math
import numpy as np

B, S, D, H = 4, 2048, 1024, 16
HD = D // H
HEADS_PER_CORE = H // 8  # 2
CD = HEADS_PER_CORE * HD  # 128 ctx dims per core

_BUILD_CACHE = {}


def _build(lengths):
    from contextlib import ExitStack
    from concourse import bacc, tile, mybir

    F32 = mybir.dt.float32
    F32R = mybir.dt.float32r
    Exp = mybir.ActivationFunctionType.Exp

    lengths = list(lengths)
    bases = np.concatenate([[0], np.cumsum(lengths)]).astype(int)
    R = int(bases[-1])  # total valid rows

    nc = bacc.Bacc("TRN2", target_bir_lowering=False, debug=False)
    xt_d = nc.declare_dram_parameter("xt", [B, D, S], F32, isOutput=False)
    wq_d = nc.declare_dram_parameter("wq", [D, CD], F32, isOutput=False)
    wk_d = nc.declare_dram_parameter("wk", [D, CD], F32, isOutput=False)
    wv_d = nc.declare_dram_parameter("wv", [D, CD], F32, isOutput=False)
    wo_d = nc.declare_dram_parameter("wo", [CD, D], F32, isOutput=False)
    tri_d = nc.declare_dram_parameter("tri", [128, 128], F32, isOutput=False)
    id_d = nc.declare_dram_parameter("ident", [128, HD], F32, isOutput=False)
    po_d = nc.declare_dram_parameter("po", [R, D], F32, isOutput=True)

    with tile.TileContext(nc) as tc, ExitStack() as ctx:
        const = ctx.enter_context(tc.tile_pool(name="const", bufs=1))
        xpool = ctx.enter_context(tc.tile_pool(name="xpool", bufs=2))
        seq = ctx.enter_context(tc.tile_pool(name="seq", bufs=2))
        epool = ctx.enter_context(tc.tile_pool(name="epool", bufs=6))
        opool = ctx.enter_context(tc.tile_pool(name="opool", bufs=3))
        small = ctx.enter_context(tc.tile_pool(name="small", bufs=4))
        # PSUM banks: s 3 + ctx 3 + pm 2 = 8
        s_ps_pool = ctx.enter_context(tc.tile_pool(name="s_ps", bufs=3, space="PSUM"))
        ctx_ps_pool = ctx.enter_context(tc.tile_pool(name="c_ps", bufs=3, space="PSUM"))
        pm_ps_pool = ctx.enter_context(tc.tile_pool(name="pm_ps", bufs=2, space="PSUM"))

        # constants / weights (loaded once)
        wq_sb = const.tile([128, 8, CD], F32R)
        wk_sb = const.tile([128, 8, CD], F32R)
        wv_sb = const.tile([128, 8, CD], F32R)
        wo_sb = const.tile([128, D], F32R)
        tri_sb = const.tile([128, 128], F32R)
        id_sb = const.tile([128, HD], F32R)
        nc.sync.dma_start(out=wq_sb, in_=wq_d.rearrange("(c p) m -> p c m", p=128).bitcast(F32R))
        nc.sync.dma_start(out=wk_sb, in_=wk_d.rearrange("(c p) m -> p c m", p=128).bitcast(F32R))
        nc.sync.dma_start(out=wv_sb, in_=wv_d.rearrange("(c p) m -> p c m", p=128).bitcast(F32R))
        nc.sync.dma_start(out=wo_sb, in_=wo_d[:, :].bitcast(F32R))
        nc.sync.dma_start(out=tri_sb, in_=tri_d[:, :].bitcast(F32R))
        nc.sync.dma_start(out=id_sb, in_=id_d[:, :].bitcast(F32R))
        onesc_f = const.tile([128, 16], F32)
        nc.vector.memset(onesc_f, 1.0)

        qT = {}
        kT = {}
        vT = {}
        v2 = {}

        def proj_unit(b, rt):
            L = lengths[b]
            r0 = rt * 512
            rn = min(512, L - r0)
            xt_t = xpool.tile([128, 8, 512], F32R, tag="xt", name=f"xt{b}_{rt}")
            nc.sync.dma_start(
                out=xt_t[:, :, :rn],
                in_=xt_d[b].rearrange("(c p) s -> p c s", p=128)[:, :, r0:r0 + rn].bitcast(F32R),
            )
            for w_sb, dstT in ((wq_sb, qT[b]), (wk_sb, kT[b]), (wv_sb, vT[b])):
                pps = pm_ps_pool.tile([128, 512], F32, tag="pm", name=f"pp{b}_{rt}")
                for kc in range(8):
                    nc.tensor.matmul(
                        pps[:, :rn], w_sb[:, kc, :], xt_t[:, kc, :rn],
                        start=(kc == 0), stop=(kc == 7),
                    )
                nc.vector.tensor_copy(dstT[:, r0:r0 + rn], pps[:, :rn])

        def vtrans_unit(b, kc):
            L = lengths[b]
            k0 = kc * 128
            kn = min(128, L - k0)
            for h2 in range(HEADS_PER_CORE):
                tp = pm_ps_pool.tile([128, HD], F32R, tag="pm", name=f"tp{b}_{kc}_{h2}")
                nc.tensor.transpose(
                    tp[:kn, :], vT[b][h2 * HD:(h2 + 1) * HD, k0:k0 + kn],
                    id_sb[h2 * HD:(h2 + 1) * HD, :],
                )
                nc.vector.tensor_copy(v2[b][:kn, kc, h2, :HD], tp[:kn, :])

        def ones_unit(b, nkc):
            for h2 in range(HEADS_PER_CORE):
                nc.vector.tensor_copy(v2[b][:, :nkc, h2, HD], onesc_f[:, :nkc])

        def batch_units(b):
            """Projection + v-transform emission units for batch b."""
            L = lengths[b]
            if L == 0:
                return []
            nrt = (L + 511) // 512
            nkc = (L + 127) // 128
            qT[b] = seq.tile([128, 2048], F32R, tag="qT", name=f"qT{b}")
            kT[b] = seq.tile([128, 2048], F32R, tag="kT", name=f"kT{b}")
            vT[b] = seq.tile([128, 2048], F32R, tag="vT", name=f"vT{b}")
            v2[b] = seq.tile([128, 16, HEADS_PER_CORE, HD + 1], F32R, tag="v2", name=f"v2{b}")
            units = [lambda b=b, nkc=nkc: ones_unit(b, nkc)]
            for rt in range(nrt):
                units.append(lambda b=b, rt=rt: proj_unit(b, rt))
                for kc in range(rt * 4, min(rt * 4 + 4, nkc)):
                    units.append(lambda b=b, kc=kc: vtrans_unit(b, kc))
            return units

        def outproj_unit(b, rt):
            L = lengths[b]
            r0 = rt * 128
            rn = min(128, L - r0)
            o_sb = opool.tile([128, D], F32, tag="o", name=f"o{b}_{rt}")
            for n in range(2):
                ops = pm_ps_pool.tile([128, 512], F32, tag="pm", name=f"op{b}_{rt}_{n}")
                nc.tensor.matmul(
                    ops[:rn, :], ctxT[b][:, r0:r0 + rn], wo_sb[:, n * 512:(n + 1) * 512],
                    start=True, stop=True,
                )
                if n == 0:
                    nc.scalar.copy(o_sb[:rn, n * 512:(n + 1) * 512], ops[:rn, :])
                else:
                    nc.vector.tensor_copy(o_sb[:rn, n * 512:(n + 1) * 512], ops[:rn, :])
            nc.sync.dma_start(
                out=po_d[int(bases[b]) + r0:int(bases[b]) + r0 + rn, :],
                in_=o_sb[:rn, :],
            )

        ctxT = {}
        from collections import deque

        # batch 0 projections up-front (dense PE warm-up)
        for u in batch_units(0):
            u()

        for b in range(B):
            L = lengths[b]
            if L == 0:
                continue
            nrt = (L + 511) // 512
            nkc = (L + 127) // 128
            ctxT[b] = seq.tile([128, 2048], F32R, tag="ctxT", name=f"ctxT{b}")

            # units of the NEXT batch to interleave into this attention phase
            nxt = deque(batch_units(b + 1)) if b + 1 < B else deque()
            n_kc_units = sum(min(nkc, (min(512, L - qt * 512) + qt * 512 + 127) // 128)
                             for qt in range(nrt))
            stride = max(1, n_kc_units // max(1, len(nxt))) if nxt else 0
            kc_count = 0

            for qt in range(nrt):
                q0 = qt * 512
                qn = min(512, L - q0)
                cps = [
                    ctx_ps_pool.tile([HD + 1, 512], F32, tag="cp", name=f"cp{b}_{qt}_{i}")
                    for i in range(HEADS_PER_CORE)
                ]
                nkc_q = min(nkc, (q0 + qn + 127) // 128)
                for kc in range(nkc_q):
                    k0 = kc * 128
                    kn = min(128, L - k0)
                    off = max(0, k0 - q0)
                    sn = qn - off
                    e_sb = []
                    for h2 in range(HEADS_PER_CORE):
                        sps = s_ps_pool.tile([128, 512], F32, tag="sp", name=f"sp{b}_{qt}_{kc}_{h2}")
                        nc.tensor.matmul(
                            sps[:kn, off:off + sn],
                            kT[b][h2 * HD:(h2 + 1) * HD, k0:k0 + kn],
                            qT[b][h2 * HD:(h2 + 1) * HD, q0 + off:q0 + qn],
                            start=True, stop=True,
                        )
                        e = epool.tile([128, 512], F32R, tag="e", name=f"e{b}_{qt}_{kc}_{h2}")
                        nc.scalar.activation(e[:kn, off:off + sn], sps[:kn, off:off + sn], Exp)
                        e_sb.append(e)
                    if k0 >= q0:  # diagonal block: mask the triangle
                        mn = min(128, sn)
                        for h2 in range(HEADS_PER_CORE):
                            nc.vector.tensor_mul(
                                e_sb[h2][:kn, off:off + mn],
                                e_sb[h2][:kn, off:off + mn],
                                tri_sb[:kn, :mn],
                            )
                    for h2 in range(HEADS_PER_CORE):
                        nc.tensor.matmul(
                            cps[h2][:, off:off + sn],
                            v2[b][:kn, kc, h2, :],
                            e_sb[h2][:kn, off:off + sn],
                            start=(kc == 0), stop=(kc == nkc_q - 1),
                        )
                    kc_count += 1
                    if nxt and stride and kc_count % stride == 0:
                        nxt.popleft()()
                # normalize: ctxT[0:64] / ctxT[64] -> ctxT_sb
                for h2 in range(HEADS_PER_CORE):
                    rs = small.tile([1, 512], F32, tag="rs", name=f"rs{b}_{qt}_{h2}")
                    nc.scalar.copy(rs[:, :qn], cps[h2][HD:HD + 1, :qn])
                    rec = small.tile([1, 512], F32, tag="rec", name=f"rec{b}_{qt}_{h2}")
                    nc.vector.reciprocal_approx_fast(out=rec[:, :qn], in_=rs[:, :qn])
                    bc_sb = small.tile([HD, 512], F32, tag="bc", name=f"bc{b}_{qt}_{h2}")
                    nc.gpsimd.partition_broadcast(bc_sb[:, :qn], rec[:, :qn])
                    nc.vector.tensor_mul(
                        ctxT[b][h2 * HD:(h2 + 1) * HD, q0:q0 + qn],
                        cps[h2][:HD, :qn],
                        bc_sb[:, :qn],
                    )
                # out-projection for the rows this q-tile covers
                for rt in range(qt * 4, min(qt * 4 + 4, nkc)):
                    outproj_unit(b, rt)
            while nxt:
                nxt.popleft()()

    nc.finalize()
    return nc, bases


def _prep_inputs(x, Wq, Wk, Wv, Wo):
    xt = np.ascontiguousarray(np.transpose(np.asarray(x, np.float32), (0, 2, 1)))
    tri = (np.arange(128)[None, :] >= np.arange(128)[:, None]).astype(np.float32)
    ident = np.concatenate([np.eye(HD, dtype=np.float32)] * 2, axis=0)
    maps = []
    for c in range(8):
        sl = slice(c * CD, (c + 1) * CD)
        maps.append({
            "xt": xt,
            "wq": np.ascontiguousarray(np.asarray(Wq, np.float32)[:, sl]) * (1.0 / math.sqrt(HD)),
            "wk": np.ascontiguousarray(np.asarray(Wk, np.float32)[:, sl]),
            "wv": np.ascontiguousarray(np.asarray(Wv, np.float32)[:, sl]),
            "wo": np.ascontiguousarray(np.asarray(Wo, np.float32)[sl, :]),
            "tri": tri,
            "ident": ident,
        })
    return maps


def run_device(x, Wq, Wk, Wv, Wo, valid_mask, trace=False):
    """Returns (partials_summed [R, D], bases, results_obj)."""
    from concourse.bass_utils import run_bass_kernel_spmd

    lengths = tuple(int(v) for v in np.asarray(valid_mask).astype(np.int64).sum(axis=1))
    if lengths not in _BUILD_CACHE:
        _BUILD_CACHE[lengths] = _build(lengths)
    nc, bases = _BUILD_CACHE[lengths]

    maps = _prep_inputs(x, Wq, Wk, Wv, Wo)
    res = run_bass_kernel_spmd(nc, maps, list(range(8)), trace=trace)
    acc = res.results[0]["po"].astype(np.float32)
    for c in range(1, 8):
        acc += res.results[c]["po"]
    return acc, bases, res


def kernel(x, Wq, Wk, Wv, Wo, bo, valid_mask):
    x = np.asarray(x)
    vm = np.asarray(valid_mask)
    acc, bases, _ = run_device(x, Wq, Wk, Wv, Wo, vm, trace=False)
    lengths = vm.astype(np.int64).sum(axis=1)
    out = np.zeros((B, S, D), dtype=np.float32)
    for b in range(B):
        L = int(lengths[b])
        if L:
            out[b, :L, :] = acc[int(bases[b]):int(bases[b]) + L, :] + np.asarray(bo, np.float32)[None, :]
    return out
